# revision 18
# baseline (speedup 1.0000x reference)
"""GCN message-passing kernel for Trainium2, distributed across 8 NeuronCores.

Model (per graph): h = (x @ W_embed) / rowsum(x); 3 GCN layers of
h = relu((sum_k h[nbr_k] / max(vl,1)) @ W_l + h @ B_l); classifier MLP + softmax.

Sharding: cores 0-3 take graph 0, cores 4-7 take graph 1; within a graph each
core owns a contiguous block of 5000 nodes.  After the embed and after each of
the first two GCN layers the per-core h block is AllGather'd (replica groups
[0-3], [4-7]) into a full-graph "table" in DRAM, from which the neighbor
gather (indirect DMA, 32 rows of H floats per node) is served.

The neighbor gather dominates: B*N*K*H*4 bytes = 655 MB per layer across the
16 DMA rings of 8 cores.  Everything else (matmuls, reductions, softmax) is
tiny by comparison and overlaps with the gather stream under Tile scheduling.
"""

import sys

if "/opt/trn_rl_repo" not in sys.path:
    sys.path.insert(0, "/opt/trn_rl_repo")

import numpy as np

import concourse.bass as bass
import concourse.bacc as bacc
import concourse.mybir as mybir
import concourse.tile as tile
from concourse.masks import make_identity
from concourse.bass_utils import run_bass_kernel_spmd

F32 = mybir.dt.float32
I32 = mybir.dt.int32
I16 = mybir.dt.int16

# Full-size problem constants
B, N, K, H, C, L = 2, 20000, 32, 128, 40, 3
N_CORES = 8
GROUP = 4                # cores per graph
N_LOC = N // GROUP       # nodes owned per core
N_TILES = (N_LOC + 127) // 128
N_PAD = N_TILES * 128

AX = mybir.AxisListType
ALU = mybir.AluOpType
ACT_F = mybir.ActivationFunctionType


def build_program(n_graph=N, n_loc=N_LOC, table_dt=F32, n_cores=N_CORES,
                  group=GROUP, k=K, h=H, c=C, n_layers=L, debug_h=False):
    """Build the SPMD Bass program (same graph on every core)."""
    n_tiles = (n_loc + 127) // 128
    n_pad = n_tiles * 128
    rg = [list(range(g * group, (g + 1) * group))
          for g in range(n_cores // group)]

    nc = bacc.Bacc("TRN2", target_bir_lowering=False, debug=False,
                   num_devices=n_cores)

    x_own = nc.dram_tensor("x_own", [n_pad, h], F32, kind="ExternalInput")
    # idx_own holds, per node-tile, the dma_gather int16 index pattern:
    # column c*8+r at partition 16a+q (replicated over a=0..7 gpsimd cores)
    # is nbr_idx[t*128 + r*16 + q, c]; stream order j = c*128 + p.
    s_per_tile = 8 * k
    idx_own = nc.dram_tensor("idx_own", [128, n_tiles * s_per_tile], I16,
                             kind="ExternalInput")
    # vl_own pre-tiled the same way: vl_own[p, t] = valid_lens[t*128 + p]
    vl_own = nc.dram_tensor("vl_own", [128, n_tiles], I32, kind="ExternalInput")
    w_embed_d = nc.dram_tensor("W_embed", [h, h], F32, kind="ExternalInput")
    gcn_w_d = nc.dram_tensor("gcn_W", [n_layers, h, h], F32, kind="ExternalInput")
    gcn_b_d = nc.dram_tensor("gcn_B", [n_layers, h, h], F32, kind="ExternalInput")
    w1_d = nc.dram_tensor("cls_W1", [h, h], F32, kind="ExternalInput")
    b1_d = nc.dram_tensor("cls_b1", [1, h], F32, kind="ExternalInput")
    w2_d = nc.dram_tensor("cls_W2", [h, c], F32, kind="ExternalInput")
    b2_d = nc.dram_tensor("cls_b2", [1, c], F32, kind="ExternalInput")
    out_d = nc.dram_tensor("out", [n_pad, c], F32, kind="ExternalOutput")
    dbg_d = None
    if debug_h:
        dbg_d = [nc.dram_tensor(f"dbg_h{i}", [n_pad, h], F32,
                                kind="ExternalOutput")
                 for i in range(n_layers + 1)]

    from contextlib import ExitStack
    with tile.TileContext(nc) as tc, ExitStack() as ctx:
        const = ctx.enter_context(tc.tile_pool(name="const", bufs=1))
        state = ctx.enter_context(tc.tile_pool(name="state", bufs=1))
        dram = ctx.enter_context(tc.tile_pool(name="dram", bufs=1, space="DRAM"))
        xp = ctx.enter_context(tc.tile_pool(name="xp", bufs=3))
        gp = ctx.enter_context(tc.tile_pool(name="gp", bufs=3))
        sp = ctx.enter_context(tc.tile_pool(name="sp", bufs=3))
        tp = ctx.enter_context(tc.tile_pool(name="tp", bufs=3))
        psT = ctx.enter_context(tc.tile_pool(name="psT", bufs=2, space="PSUM"))
        psM = ctx.enter_context(tc.tile_pool(name="psM", bufs=2, space="PSUM"))

        # ---- constants ----
        ident = const.tile([128, 128], F32, tag="ident")
        make_identity(nc, ident[:])
        ones_row = const.tile([1, 128], F32, tag="ones_row")
        nc.gpsimd.memset(ones_row[:], 1.0)

        w_emb = const.tile([h, h], F32, tag="w_emb")
        nc.sync.dma_start(out=w_emb[:], in_=w_embed_d[:])
        wt = const.tile([h, n_layers * h], F32, tag="wt")
        bt = const.tile([h, n_layers * h], F32, tag="bt")
        for l in range(n_layers):
            nc.sync.dma_start(out=wt[:, l * h:(l + 1) * h], in_=gcn_w_d[l])
            nc.sync.dma_start(out=bt[:, l * h:(l + 1) * h], in_=gcn_b_d[l])
        w1 = const.tile([h, h], F32, tag="w1")
        nc.sync.dma_start(out=w1[:], in_=w1_d[:])
        b1 = const.tile([1, h], F32, tag="b1")
        nc.sync.dma_start(out=b1[:], in_=b1_d[:])
        w2 = const.tile([h, c], F32, tag="w2")
        nc.sync.dma_start(out=w2[:], in_=w2_d[:])
        b2 = const.tile([1, c], F32, tag="b2")
        nc.sync.dma_start(out=b2[:], in_=b2_d[:])

        # ---- persistent state ----
        idx_all = state.tile([128, n_tiles * s_per_tile], I16, tag="idx_all")
        nc.sync.dma_start(out=idx_all[:], in_=idx_own[:])
        vl_i = state.tile([128, n_tiles], I32, tag="vl_i")
        nc.sync.dma_start(out=vl_i[:], in_=vl_own[:])
        vl_f = state.tile([128, n_tiles], F32, tag="vl_f")
        nc.vector.tensor_copy(out=vl_f[:], in_=vl_i[:])
        nc.vector.tensor_scalar_max(out=vl_f[:], in0=vl_f[:], scalar1=1.0)
        inv_vl = state.tile([128, n_tiles], F32, tag="inv_vl")
        nc.vector.reciprocal(out=inv_vl[:], in_=vl_f[:])

        hs = [state.tile([128, h], F32, tag=f"hs{t}", name=f"hs{t}")
              for t in range(n_tiles)]

        # per-layer gather tables (embed output + after layers 0..n_layers-2)
        hg_own = [dram.tile([n_loc, h], table_dt, name=f"hg_own{l}")
                  for l in range(n_layers)]
        tables = [dram.tile([n_graph, h], table_dt, name=f"table{l}")
                  for l in range(n_layers)]

        def write_table(lay, t, src_sbuf):
            rows = min(128, n_loc - t * 128)
            if rows <= 0:
                return
            if table_dt == F32:
                nc.sync.dma_start(out=hg_own[lay][t * 128:t * 128 + rows, :],
                                  in_=src_sbuf[:rows, :])
            else:
                cast = sp.tile([128, h], table_dt, tag="cast16")
                nc.vector.tensor_copy(out=cast[:], in_=src_sbuf[:])
                nc.sync.dma_start(out=hg_own[lay][t * 128:t * 128 + rows, :],
                                  in_=cast[:rows, :])

        # ---- embed: h = (x / rowsum(x)) @ W_embed ----
        for t in range(n_tiles):
            x_t = xp.tile([128, h], F32, tag="x_t")
            nc.sync.dma_start(out=x_t[:], in_=x_own[t * 128:(t + 1) * 128, :])
            rs = sp.tile([128, 1], F32, tag="rs")
            nc.vector.reduce_sum(out=rs[:], in_=x_t[:], axis=AX.X)
            irs = sp.tile([128, 1], F32, tag="irs")
            nc.vector.reciprocal(out=irs[:], in_=rs[:])
            nc.vector.tensor_scalar_mul(out=x_t[:], in0=x_t[:], scalar1=irs[:])
            pt = psT.tile([128, 128], F32, tag="ptA")
            nc.tensor.transpose(out=pt[:], in_=x_t[:], identity=ident[:])
            xT = tp.tile([128, 128], F32, tag="tA")
            nc.vector.tensor_copy(out=xT[:], in_=pt[:])
            mm = psM.tile([128, h], F32, tag="mm")
            nc.tensor.matmul(out=mm[:], lhsT=xT[:], rhs=w_emb[:],
                             start=True, stop=True)
            nc.vector.tensor_copy(out=hs[t][:], in_=mm[:])
            write_table(0, t, hs[t])
            if debug_h:
                nc.sync.dma_start(out=dbg_d[0][t * 128:(t + 1) * 128, :],
                                  in_=hs[t][:])

        nc.gpsimd.collective_compute(
            "AllGather", ALU.bypass, replica_groups=rg,
            ins=[hg_own[0].opt()], outs=[tables[0].opt()])

        # ---- GCN layers ----
        for l in range(n_layers):
            for t in range(n_tiles):
                g_t = gp.tile([128, k * h], table_dt, tag="g_t")
                nc.gpsimd.dma_gather(
                    out_ap=g_t[:].rearrange("p (c e) -> p c e", e=h),
                    in_ap=tables[l],
                    idxs_ap=idx_all[:, t * s_per_tile:(t + 1) * s_per_tile],
                    num_idxs=128 * k,
                    num_idxs_reg=128 * k,
                    elem_size=h,
                    single_packet=False)
                w = k * h // 2
                while w > h:
                    nc.vector.tensor_tensor(out=g_t[:, :w], in0=g_t[:, :w],
                                            in1=g_t[:, w:2 * w], op=ALU.add)
                    w //= 2
                nsum = sp.tile([128, h], F32, tag="nsum")
                nc.vector.tensor_tensor(out=nsum[:], in0=g_t[:, :h],
                                        in1=g_t[:, h:2 * h], op=ALU.add)
                nc.vector.tensor_scalar_mul(out=nsum[:], in0=nsum[:],
                                            scalar1=inv_vl[:, t:t + 1])
                ptA = psT.tile([128, 128], F32, tag="ptA")
                nc.tensor.transpose(out=ptA[:], in_=nsum[:], identity=ident[:])
                nT = tp.tile([128, 128], F32, tag="tA")
                nc.vector.tensor_copy(out=nT[:], in_=ptA[:])
                ptB = psT.tile([128, 128], F32, tag="ptB")
                nc.tensor.transpose(out=ptB[:], in_=hs[t][:], identity=ident[:])
                hT = tp.tile([128, 128], F32, tag="tB")
                nc.vector.tensor_copy(out=hT[:], in_=ptB[:])
                mm = psM.tile([128, h], F32, tag="mm")
                nc.tensor.matmul(out=mm[:], lhsT=nT[:],
                                 rhs=wt[:, l * h:(l + 1) * h],
                                 start=True, stop=False)
                nc.tensor.matmul(out=mm[:], lhsT=hT[:],
                                 rhs=bt[:, l * h:(l + 1) * h],
                                 start=False, stop=True)
                nc.scalar.activation(out=hs[t][:], in_=mm[:], func=ACT_F.Relu)
                if l < n_layers - 1:
                    write_table(l + 1, t, hs[t])
                if debug_h:
                    nc.sync.dma_start(
                        out=dbg_d[l + 1][t * 128:(t + 1) * 128, :],
                        in_=hs[t][:])
            if l < n_layers - 1:
                nc.gpsimd.collective_compute(
                    "AllGather", ALU.bypass, replica_groups=rg,
                    ins=[hg_own[l + 1].opt()], outs=[tables[l + 1].opt()])

        # ---- classifier + softmax ----
        for t in range(n_tiles):
            ptA = psT.tile([128, 128], F32, tag="ptA")
            nc.tensor.transpose(out=ptA[:], in_=hs[t][:], identity=ident[:])
            hT = tp.tile([128, 128], F32, tag="tA")
            nc.vector.tensor_copy(out=hT[:], in_=ptA[:])
            mm = psM.tile([128, h], F32, tag="mm")
            nc.tensor.matmul(out=mm[:], lhsT=hT[:], rhs=w1[:],
                             start=True, stop=False)
            nc.tensor.matmul(out=mm[:], lhsT=ones_row[:], rhs=b1[:],
                             start=False, stop=True)
            z = sp.tile([128, h], F32, tag="z")
            nc.scalar.activation(out=z[:], in_=mm[:], func=ACT_F.Relu)
            ptB = psT.tile([128, 128], F32, tag="ptB")
            nc.tensor.transpose(out=ptB[:], in_=z[:], identity=ident[:])
            zT = tp.tile([128, 128], F32, tag="tB")
            nc.vector.tensor_copy(out=zT[:], in_=ptB[:])
            mml = psM.tile([128, c], F32, tag="mml")
            nc.tensor.matmul(out=mml[:], lhsT=zT[:], rhs=w2[:],
                             start=True, stop=False)
            nc.tensor.matmul(out=mml[:], lhsT=ones_row[:], rhs=b2[:],
                             start=False, stop=True)
            negmax = sp.tile([128, 1], F32, tag="negmax")
            nc.vector.reduce_max(out=negmax[:], in_=mml[:], axis=AX.X,
                                 negate=True)
            e_t = sp.tile([128, c], F32, tag="e_t")
            sume = sp.tile([128, 1], F32, tag="sume")
            nc.scalar.activation(out=e_t[:], in_=mml[:], func=ACT_F.Exp,
                                 bias=negmax[:], accum_out=sume[:])
            isum = sp.tile([128, 1], F32, tag="isum")
            nc.vector.reciprocal(out=isum[:], in_=sume[:])
            p_t = sp.tile([128, c], F32, tag="p_t")
            nc.vector.tensor_scalar_mul(out=p_t[:], in0=e_t[:], scalar1=isum[:])
            nc.sync.dma_start(out=out_d[t * 128:(t + 1) * 128, :], in_=p_t[:])

    nc.compile()
    return nc


def make_in_maps(vertex_feat, neighbors_idx, valid_lens, W_embed, gcn_W,
                 gcn_B, cls_W1, cls_b1, cls_W2, cls_b2,
                 n_loc=N_LOC, group=GROUP, n_cores=N_CORES, k=K, h=H):
    n_tiles = (n_loc + 127) // 128
    n_pad = n_tiles * 128
    shared = dict(
        W_embed=np.ascontiguousarray(W_embed, np.float32),
        gcn_W=np.ascontiguousarray(gcn_W, np.float32),
        gcn_B=np.ascontiguousarray(gcn_B, np.float32),
        cls_W1=np.ascontiguousarray(cls_W1, np.float32),
        cls_b1=np.ascontiguousarray(cls_b1, np.float32).reshape(1, h),
        cls_W2=np.ascontiguousarray(cls_W2, np.float32),
        cls_b2=np.ascontiguousarray(cls_b2, np.float32).reshape(1, -1),
    )
    in_maps = []
    for cid in range(n_cores):
        g, s = cid // group, cid % group
        rows = slice(s * n_loc, (s + 1) * n_loc)
        x = np.ones((n_pad, h), np.float32)
        x[:n_loc] = vertex_feat[g, rows]
        idx = np.zeros((n_pad, k), np.int32)
        idx[:n_loc] = neighbors_idx[g, rows]
        # dma_gather wrapped layout: pattern[t, q, c*8+r] = idx[t*128+r*16+q, c],
        # replicated over the 8 gpsimd cores (partition groups of 16).
        pat = idx.reshape(n_tiles, 8, 16, k).transpose(0, 2, 3, 1)
        pat = pat.reshape(n_tiles, 16, k * 8)
        idx_t = np.tile(pat, (1, 8, 1)).transpose(1, 0, 2)
        idx_t = np.ascontiguousarray(idx_t.reshape(128, -1).astype(np.int16))
        vl = np.ones((n_pad,), np.int32)
        vl[:n_loc] = valid_lens[g, rows]
        vl_t = np.ascontiguousarray(vl.reshape(n_tiles, 128).T)
        in_maps.append(dict(x_own=x, idx_own=idx_t, vl_own=vl_t, **shared))
    return in_maps


_PROG_CACHE = {}


def _assemble(results):
    per_core = [results[cid]["out"][:N_LOC] for cid in range(N_CORES)]
    return np.stack([
        np.concatenate(per_core[g * GROUP:(g + 1) * GROUP], axis=0)
        for g in range(B)
    ]).astype(np.float32)


def kernel(vertex_feat, neighbors_idx, valid_lens, W_embed, gcn_W, gcn_B,
           cls_W1, cls_b1, cls_W2, cls_b2):
    key = "full"
    if key not in _PROG_CACHE:
        _PROG_CACHE[key] = build_program()
    nc = _PROG_CACHE[key]
    in_maps = make_in_maps(vertex_feat, neighbors_idx, valid_lens, W_embed,
                           gcn_W, gcn_B, cls_W1, cls_b1, cls_W2, cls_b2)
    res = run_bass_kernel_spmd(nc, in_maps, list(range(N_CORES)))
    return _assemble(res.results)


def run_timed(nc, in_maps, n_cores, n_iters=10):
    """Clone of bass2jax.run_bass_via_pjrt's multi-core branch that keeps
    the jitted callable, so repeated executions can be wall-clocked on
    device-resident inputs.  Returns (per-core results, sorted times_s)."""
    import time
    import jax
    from jax.sharding import Mesh, PartitionSpec, NamedSharding
    from jax.experimental.shard_map import shard_map
    import concourse.mybir as mybir_
    from concourse import bass2jax

    bass2jax.install_neuronx_cc_hook()
    partition_name = (nc.partition_id_tensor.name
                      if nc.partition_id_tensor else None)
    in_names, out_names, out_avals, zero_outs = [], [], [], []
    for alloc in nc.m.functions[0].allocations:
        if not isinstance(alloc, mybir_.MemoryLocationSet):
            continue
        name = alloc.memorylocations[0].name
        if alloc.kind == "ExternalInput":
            if name != partition_name:
                in_names.append(name)
        elif alloc.kind == "ExternalOutput":
            out_names.append(name)
            shape = tuple(alloc.tensor_shape)
            dtype = mybir_.dt.np(alloc.dtype)
            out_avals.append(jax.core.ShapedArray(shape, dtype))
            zero_outs.append(np.zeros(shape, dtype))
    n_params = len(in_names)
    n_outs = len(out_avals)
    in_names_all = in_names + out_names
    if partition_name is not None:
        in_names_all.append(partition_name)

    donate = tuple(range(n_params, n_params + n_outs))

    def _body(*args):
        operands = list(args)
        if partition_name is not None:
            operands.append(bass2jax.partition_id_tensor())
        outs = bass2jax._bass_exec_p.bind(
            *operands,
            out_avals=tuple(out_avals),
            in_names=tuple(in_names_all),
            out_names=tuple(out_names),
            lowering_input_output_aliases=(),
            sim_require_finite=True,
            sim_require_nnan=True,
            nc=nc,
        )
        return tuple(outs)

    devices = jax.devices()[:n_cores]
    mesh = Mesh(np.asarray(devices), ("core",))
    sharded = jax.jit(
        shard_map(_body, mesh=mesh,
                  in_specs=(PartitionSpec("core"),) * (n_params + n_outs),
                  out_specs=(PartitionSpec("core"),) * len(out_names),
                  check_rep=False),
        donate_argnums=donate, keep_unused=True)

    per_core = [[np.asarray(m[name]) for name in in_names] for m in in_maps]
    concat_in = [np.concatenate([per_core[c][i] for c in range(n_cores)], axis=0)
                 for i in range(n_params)]
    concat_zeros = [np.zeros((n_cores * z.shape[0], *z.shape[1:]), z.dtype)
                    for z in zero_outs]

    sh = NamedSharding(mesh, PartitionSpec("core"))
    dev_in = [jax.device_put(a, sh) for a in concat_in]
    jax.block_until_ready(dev_in)

    out_arrs = sharded(*dev_in, *[jax.device_put(z, sh) for z in concat_zeros])
    jax.block_until_ready(out_arrs)
    results = [
        {name: np.asarray(out_arrs[i]).reshape(n_cores, *out_avals[i].shape)[c]
         for i, name in enumerate(out_names)}
        for c in range(n_cores)
    ]

    times = []
    for _ in range(n_iters):
        zdev = [jax.device_put(z, sh) for z in concat_zeros]
        jax.block_until_ready(zdev)
        t0 = time.perf_counter()
        outs = sharded(*dev_in, *zdev)
        jax.block_until_ready(outs)
        times.append(time.perf_counter() - t0)
    return results, sorted(times)


def build_noop_program(n_cores=N_CORES):
    """Tiny program for estimating the axon dispatch floor."""
    nc = bacc.Bacc("TRN2", target_bir_lowering=False, debug=False,
                   num_devices=n_cores)
    a = nc.dram_tensor("a", [128, 128], F32, kind="ExternalInput")
    o = nc.dram_tensor("out", [128, 128], F32, kind="ExternalOutput")
    from contextlib import ExitStack
    with tile.TileContext(nc) as tc, ExitStack() as ctx:
        sb = ctx.enter_context(tc.tile_pool(name="sb", bufs=1))
        t = sb.tile([128, 128], F32, tag="t")
        nc.sync.dma_start(out=t[:], in_=a[:])
        nc.sync.dma_start(out=o[:], in_=t[:])
    nc.compile()
    return nc
